# revision 5
# baseline (speedup 1.0000x reference)
"""Trainium2 Bass kernel for causal multi-head attention block.

Module: qkv = x @ W_attn + b_attn; causal softmax((q k^T)/sqrt(C)); y = (attn @ v) @ W_proj + b_proj
Shapes (hardcoded): x [8, 1024, 768], W_attn [768, 2304], W_proj [768, 768], H=12, D=64.

Sharding: data parallel over batch — core b computes batch element b (B == n_cores == 8).
No collectives; host scatters x and gathers y.

Per-core dataflow (all matmul operands float32r = 4-byte fp32 at full PE rate):
  1. x [1024,768] -> PE-transpose -> xT [768,1024]          (6 chunks of [128,1024])
  2. QT,KT [c,1024] = (W_attn chunk).T @ xT   (qkv transposed; W stationary)
     V [1024, 780]  = xT.T @ W_attn_vcols    (natural layout, +1.0 column per head)
  3. per head h, per 512-query group g:
       scoresT[tk,tq] = KT_h_tile.T @ QT_h     (causal-tight: tq >= tk tiles only)
       expT = Exp(scoresT * 1/sqrt(768))       (ScalarE, PSUM->SBUF)
       diag 128x128 block *= upper-tri mask
       av[65, tq]  += [V_h | 1].T @ expT       (row 64 = sum of exp = softmax denom)
       yT_h = av[0:64] * broadcast(1/av[64])   (recip on DVE, broadcast via K=1 matmul)
  4. out[tq, c] = (yT chunk).T @ W_proj chunk  (+bias), DMA natural rows to DRAM
"""

import math

import numpy as np

import concourse.bass as bass
import concourse.mybir as mybir
from concourse import bacc, tile
from concourse.bass_utils import run_bass_kernel_spmd
from concourse.masks import make_identity, make_upper_triangular

N_CORES = 8
B, T, C = 8, 1024, 768
H, D = 12, 64
C3 = 3 * C
NT = T // 128          # 8 token tiles
NK = C // 128          # 6 contraction chunks
NG = T // 512          # 2 query groups of 512
SCALE = 1.0 / math.sqrt(C)

F32 = mybir.dt.float32
F32R = mybir.dt.float32r
MULT = mybir.AluOpType.mult
EXP = mybir.ActivationFunctionType.Exp


def _build_nc(has_battn: bool, has_bproj: bool, num_devices: int):
    nc = bacc.Bacc(
        "TRN2", target_bir_lowering=False, debug=False, num_devices=num_devices
    )

    x_d = nc.dram_tensor("x", (T, C), F32R, kind="ExternalInput")
    wa_d = nc.dram_tensor("W_attn", (C, C3), F32R, kind="ExternalInput")
    wp_d = nc.dram_tensor("W_proj", (C, C), F32R, kind="ExternalInput")
    ba_d = bp_d = None
    if has_battn:
        ba_d = nc.dram_tensor("b_attn", (C3,), F32, kind="ExternalInput")
    if has_bproj:
        bp_d = nc.dram_tensor("b_proj", (C,), F32, kind="ExternalInput")
    y_d = nc.dram_tensor("y", (T, C), F32, kind="ExternalOutput")

    with tile.TileContext(nc) as tc:
        with (
            tc.tile_pool(name="sb", bufs=1) as sb,
            tc.tile_pool(name="ps", bufs=1, space="PSUM") as ps,
        ):
            _emit(nc, sb, ps, x_d, wa_d, wp_d, ba_d, bp_d, y_d)
    nc.compile()
    return nc


def _emit(nc, sb, ps, x_d, wa_d, wp_d, ba_d, bp_d, y_d):
    def st(shape, tag, dtype=F32R, name=None):
        return sb.tile(list(shape), dtype, tag=tag, bufs=_SB_BUFS[tag],
                       name=name or tag)

    def pt(shape, tag, dtype=F32, name=None):
        return ps.tile(list(shape), dtype, tag=tag, bufs=_PS_BUFS[tag],
                       name=name or tag)

    # --- constants ---
    # memset/affine_select can't encode float32r, and the BIR verifier demands
    # f32r matmul operands come from f32r-rounding producers — so constants are
    # built in plain f32 and DVE-copied into f32r tiles.
    ident32 = st([128, 128], "ident32", dtype=F32)
    make_identity(nc, ident32[:])
    ident = st([128, 128], "ident")
    nc.vector.tensor_copy(ident[:], ident32[:])
    tri32 = st([128, 128], "tri32", dtype=F32)
    make_upper_triangular(nc, tri32[:], val=1.0, diag=True)  # tri[r,c]=1 iff c>=r
    tri = st([128, 128], "tri")
    nc.vector.tensor_copy(tri[:], tri32[:])
    ones32 = st([1, 128], "ones32", dtype=F32)
    nc.gpsimd.memset(ones32[:], 1.0)
    ones = st([1, 128], "ones")
    nc.vector.tensor_copy(ones[:], ones32[:])
    vcol32 = st([128, H], "vcol32", dtype=F32)
    nc.gpsimd.memset(vcol32[:], 1.0)

    bqk = []
    bv_row = bp_row = None
    if ba_d is not None:
        for m in range(12):  # Q,K outchan tiles 0..1535
            bt = st([128, 1], "bqk", dtype=F32)
            nc.sync.dma_start(bt[:], ba_d.ap()[m * 128 : (m + 1) * 128])
            bqk.append(bt)
        bv_row = st([1, C], "bvrow", dtype=F32)
        nc.sync.dma_start(bv_row[:], ba_d.ap()[2 * C : 3 * C])
    if bp_d is not None:
        bp_row = st([1, C], "bprow", dtype=F32)
        nc.sync.dma_start(bp_row[:], bp_d.ap())

    # --- persistent activations ---
    xT = [st([128, T], "xT", name=f"xT{i}") for i in range(NK)]
    qT = [st([128, T], "qkT", name=f"qT{i}") for i in range(NK)]
    kT = [st([128, T], "qkT", name=f"kT{i}") for i in range(NK)]
    # V natural layout with a ones column per head: [t, 12*(64+1)]
    vt = [st([128, H * (D + 1)], "v", name=f"v{i}") for i in range(NT)]
    yT = [st([128, T], "yT", name=f"yT{i}") for i in range(NK)]

    # --- phase 0: load x, transpose to xT ---
    for t in range(NT):
        xin = st([128, C], "xin")
        nc.sync.dma_start(xin[:], x_d.ap()[t * 128 : (t + 1) * 128, :])
        for c in range(NK):
            ptt = pt([128, 128], "trbc", dtype=F32R)
            nc.tensor.transpose(ptt[:], xin[:, c * 128 : (c + 1) * 128], ident[:])
            nc.vector.tensor_copy(xT[c][:, t * 128 : (t + 1) * 128], ptt[:])

    # --- phase 1a: QT / KT (chunk c covers heads 2c, 2c+1) ---
    def emit_qk(c):
        for dst, m in ((qT[c], c), (kT[c], c + NK)):
            was = []
            for k in range(NK):
                wa = st([128, 128], "wa")
                nc.sync.dma_start(
                    wa[:],
                    wa_d.ap()[k * 128 : (k + 1) * 128, m * 128 : (m + 1) * 128],
                )
                was.append(wa)
            for g in range(NG):
                acc = pt([128, 512], "qkv")
                for k in range(NK):
                    nc.tensor.matmul(
                        acc[:],
                        was[k][:],
                        xT[k][:, g * 512 : (g + 1) * 512],
                        start=(k == 0),
                        stop=(k == NK - 1),
                    )
                dst_ap = dst[:, g * 512 : (g + 1) * 512]
                if ba_d is not None:
                    nc.vector.tensor_scalar_add(dst_ap, acc[:], bqk[m][:])
                else:
                    nc.vector.tensor_copy(dst_ap, acc[:])

    # --- phase 1b: V natural [t, heads*(D+1)] ---
    def emit_v():
        for t in range(NT):
            nc.vector.tensor_copy(
                vt[t][:].rearrange("p (h e) -> p h e", e=D + 1)[:, :, D : D + 1],
                vcol32[:].rearrange("p (h e) -> p h e", e=1),
            )
        for co, w in ((0, 512), (512, 256)):
            wvs = []
            for k in range(NK):
                wv = st([128, w], "wv")
                nc.sync.dma_start(
                    wv[:],
                    wa_d.ap()[k * 128 : (k + 1) * 128, 2 * C + co : 2 * C + co + w],
                )
                wvs.append(wv)
            for t in range(NT):
                acc = pt([128, w], "qkv")
                for k in range(NK):
                    nc.tensor.matmul(
                        acc[:],
                        xT[k][:, t * 128 : (t + 1) * 128],
                        wvs[k][:],
                        start=(k == 0),
                        stop=(k == NK - 1) and ba_d is None,
                    )
                if ba_d is not None:
                    nc.tensor.matmul(
                        acc[:],
                        ones32[0:1, 0:128],
                        bv_row[0:1, co : co + w],
                        start=False,
                        stop=True,
                    )
                dst = vt[t][:].rearrange("p (h e) -> p h e", e=D + 1)
                h0 = co // D
                nc.vector.tensor_copy(
                    dst[:, h0 : h0 + w // D, 0:D],
                    acc[:].rearrange("p (h e) -> p h e", e=D),
                )

    # --- phase 2: attention for one head ---
    def emit_head(h):
        ch, off = h // 2, (h % 2) * 64
        for g in range(NG):
            jmax = 4 * g + 3
            av = pt([D + 1, 512], "av")
            for j in range(jmax + 1):
                ow = max(j * 128 - g * 512, 0)
                nw = 512 - ow
                base = g * 512 + ow
                sc = pt([128, nw], "sc")
                nc.tensor.matmul(
                    sc[:],
                    kT[ch][off : off + 64, j * 128 : (j + 1) * 128],
                    qT[ch][off : off + 64, base : base + nw],
                    start=True,
                    stop=True,
                )
                e = st([128, nw], "expt")
                nc.scalar.activation(e[:], sc[:], EXP, scale=SCALE)
                if j >= 4 * g:  # diagonal block: keep tq >= tk only
                    nc.vector.tensor_tensor(e[:, 0:128], e[:, 0:128], tri[:], op=MULT)
                nc.tensor.matmul(
                    av[:, ow : ow + nw],
                    vt[j][:].rearrange("p (h e) -> p h e", e=D + 1)[:, h, :],
                    e[:],
                    start=(j == 0),
                    stop=(j == jmax),
                )
            rc = st([1, 512], "rc")
            with nc.allow_low_precision("float32r is 4-byte fp32"):
                nc.vector.reciprocal(rc[:], av[D : D + 1, :])
            bc_ps = pt([64, 512], "trbc")
            nc.tensor.matmul(bc_ps[:], ones[0:1, 0:64], rc[:], start=True, stop=True)
            bc = st([64, 512], "bc")
            nc.scalar.copy(bc[:], bc_ps[:])
            nc.vector.tensor_tensor(
                yT[ch][off : off + 64, g * 512 : (g + 1) * 512],
                av[0:D, :],
                bc[:],
                op=MULT,
            )

    # --- phase 3: output projection, natural layout out ---
    def emit_proj():
        wps = []
        for k in range(NK):
            wp = st([128, C], "wp")
            nc.sync.dma_start(wp[:], wp_d.ap()[k * 128 : (k + 1) * 128, :])
            wps.append(wp)
        for t in range(NT):
            osb = st([128, C], "osb", dtype=F32)
            for co, w in ((0, 512), (512, 256)):
                acc = pt([128, w], "qkv")
                for k in range(NK):
                    nc.tensor.matmul(
                        acc[:],
                        yT[k][:, t * 128 : (t + 1) * 128],
                        wps[k][:, co : co + w],
                        start=(k == 0),
                        stop=(k == NK - 1) and bp_d is None,
                    )
                if bp_d is not None:
                    nc.tensor.matmul(
                        acc[:],
                        ones32[0:1, 0:128],
                        bp_row[0:1, co : co + w],
                        start=False,
                        stop=True,
                    )
                nc.vector.tensor_copy(osb[:, co : co + w], acc[:])
            nc.sync.dma_start(y_d.ap()[t * 128 : (t + 1) * 128, :], osb[:])

    emit_qk(0)
    emit_v()
    for c in range(NK):
        if c + 1 < NK:
            emit_qk(c + 1)
        emit_head(2 * c)
        emit_head(2 * c + 1)
    emit_proj()


# pool buffer counts, patched onto tile_pool via tags at tile() time
_SB_BUFS = {
    "ident": 1, "ident32": 1, "tri": 1, "tri32": 1, "ones": 1, "ones32": 1, "vcol32": 1, "bqk": 12, "bvrow": 1, "bprow": 1,
    "xT": 6, "qkT": 12, "v": 8, "yT": 6,
    "xin": 2, "wa": 12, "wv": 6, "expt": 5, "rc": 2, "bc": 2, "wp": 6, "osb": 2,
}
_PS_BUFS = {"trbc": 2, "qkv": 2, "sc": 2, "av": 2}


_NC_CACHE = {}


def _get_nc(has_battn, has_bproj, num_devices=N_CORES):
    key = (has_battn, has_bproj, num_devices)
    if key not in _NC_CACHE:
        _NC_CACHE[key] = _build_nc(has_battn, has_bproj, num_devices)
    return _NC_CACHE[key]


def kernel(x, W_attn, b_attn, W_proj, b_proj):
    x = np.ascontiguousarray(np.asarray(x, dtype=np.float32))
    W_attn = np.ascontiguousarray(np.asarray(W_attn, dtype=np.float32))
    W_proj = np.ascontiguousarray(np.asarray(W_proj, dtype=np.float32))
    b_attn = np.asarray(b_attn, dtype=np.float32)
    b_proj = np.asarray(b_proj, dtype=np.float32)
    has_battn = bool(np.any(b_attn != 0.0))
    has_bproj = bool(np.any(b_proj != 0.0))

    nc = _get_nc(has_battn, has_bproj)
    in_maps = []
    for b in range(N_CORES):
        m = {"x": x[b], "W_attn": W_attn, "W_proj": W_proj}
        if has_battn:
            m["b_attn"] = b_attn
        if has_bproj:
            m["b_proj"] = b_proj
        in_maps.append(m)
    res = run_bass_kernel_spmd(nc, in_maps, core_ids=list(range(N_CORES)))
    return np.stack([res.results[b]["y"] for b in range(N_CORES)]).astype(np.float32)
